# revision 2
# baseline (speedup 1.0000x reference)
"""2-layer GCN on 8 Trainium2 NeuronCores — v2 (transform-first ELL gather).

Strategy:
  - Nodes ranked by in-degree (desc), dealt round-robin to 8 cores; core c
    owns dst nodes rank%8==c. Per-core ELL: tile t covers 128 dst nodes,
    J[t] slot-columns; gathers via gpsimd.dma_gather (int16 idx).
  - GCN norm factorizes: out = dinv ⊙ ((A+I)(dinv ⊙ h)); aggregation is an
    unweighted gather+sum of pre-scaled rows.
  - BOTH layers are transform-first: the gather tables hold 64-wide
    dinv-scaled transformed features in f16, stored as [rows, 128] with
    cols 64:128 unused junk (gather elements must be 256B multiples).
    Tree-reduce touches only cols 0:64 (strided views).
  - L1 table xs = dinv ⊙ (x @ W1): computed sharded (1/8 rows per core),
    assembled with AllGather. L2 table u2 = dinv ⊙ (relu(h1) @ W2): per-core
    shard + AllGather (as baseline).
  - int16 idx range handled with 2 overlapping 32768-row windows
    (bases 0 and v-32768); middle rows balanced per dst node.
"""

import sys

sys.path.insert(0, "/opt/trn_rl_repo")

import numpy as np

P = 128
CORES = 8
WIN = 32768
IN_C = 128
HID_C = 64
OUT_C = 64
MAX_J_CHUNK = 64  # 128*64 = 8192 idx / dma_gather instruction

_np_f32 = np.float32


def _cfg(n):
    npc = -(-n // CORES)
    tiles = -(-npc // P)
    jpad = tiles * P
    v1 = -(-(n + 1) // (P * CORES)) * P * CORES  # natural table rows (row0=zero)
    v2 = CORES * jpad
    return dict(n=n, npc=npc, tiles=tiles, jpad=jpad, v1=v1, v2=v2)


def _windows(v):
    if v <= WIN:
        return [0]
    return [0, v - WIN]


def _cumcount(key):
    """Occurrence index of each element within its key-group (stable)."""
    order = np.argsort(key, kind="stable")
    ks = key[order]
    grp_start = np.r_[0, np.flatnonzero(np.diff(ks)) + 1]
    seg = np.searchsorted(grp_start, np.arange(len(ks)), side="right") - 1
    q = np.arange(len(ks)) - grp_start[seg]
    out = np.empty(len(ks), dtype=np.int64)
    out[order] = q
    return out


def _assign_windows2(nid, r, bases):
    """2-window assignment: rows only in A -> 0, only in B -> 1, overlap
    rows balanced per node to equalize per-node A/B counts."""
    if len(bases) == 1:
        return np.zeros(len(nid), dtype=np.int64)
    b0, b1 = bases
    in_a_only = r < b1
    in_b_only = r >= b0 + WIN
    mid = ~(in_a_only | in_b_only)
    nmax = int(nid.max()) + 1
    a = np.bincount(nid[in_a_only], minlength=nmax)
    b = np.bincount(nid[in_b_only], minlength=nmax)
    m = np.bincount(nid[mid], minlength=nmax)
    # send x of the m middle slots to A: minimize max(a+x, b+m-x)
    x = np.clip((b + m - a + 1) // 2, 0, m)
    q = _cumcount(nid * 2 + mid.astype(np.int64))  # cumcount among mid slots
    win = np.zeros(len(nid), dtype=np.int64)
    win[in_b_only] = 1
    win[mid] = (q[mid] >= x[nid[mid]]).astype(np.int64)
    ok = (r >= np.array(bases)[win]) & (r < np.array(bases)[win] + WIN)
    assert ok.all(), "window assignment out of range"
    return win


def _build_layer_tables(slot_nid, slot_r, bases, zero_rows, t_of, p_of, c_of, tiles):
    """Per-core int16 idx arrays + shape info for one layer.

    Returns (J [tiles,nwin], Foff [tiles,nwin], F, idx16 [CORES,16,F])."""
    nwin = len(bases)
    win = _assign_windows2(slot_nid, slot_r, bases)
    jslot = _cumcount(slot_nid * 2 + win)

    t_slot = t_of[slot_nid]
    J = np.zeros((tiles, nwin), dtype=np.int64)
    np.maximum.at(J, (t_slot, win), jslot + 1)

    blk = 8 * J  # idx cols per (tile, window) block
    Foff = np.zeros((tiles, nwin), dtype=np.int64)
    flat = blk.reshape(-1)
    Foff.reshape(-1)[1:] = np.cumsum(flat)[:-1]
    F = int(flat.sum())

    idx16 = np.empty((CORES, 16, F), dtype=np.int16)
    for t in range(tiles):
        for w in range(nwin):
            if J[t, w] == 0:
                continue
            fo, ln = Foff[t, w], blk[t, w]
            idx16[:, :, fo : fo + ln] = np.int16(zero_rows[w] - bases[w])
    p_slot = p_of[slot_nid]
    c_slot = c_of[slot_nid]
    i = jslot * P + p_slot
    col = Foff[t_slot, win] + i // 16
    row = i % 16
    val = (slot_r - np.array(bases)[win]).astype(np.int16)
    idx16[c_slot, row, col] = val
    return J, Foff, F, idx16


def _prep(x, edge_index):
    n = x.shape[0]
    cfg = _cfg(n)
    npc, tiles, jpad, v1, v2 = (
        cfg["npc"],
        cfg["tiles"],
        cfg["jpad"],
        cfg["v1"],
        cfg["v2"],
    )
    assert npc < jpad, "shard needs pad rows (zero-gather targets)"
    assert v1 % (P * CORES) == 0
    src = np.asarray(edge_index[0], dtype=np.int64)
    dst = np.asarray(edge_index[1], dtype=np.int64)
    deg = np.bincount(dst, minlength=n) + 1  # in-degree + self-loop

    order = np.argsort(-deg, kind="stable")  # rank -> node
    rank_of = np.empty(n, dtype=np.int64)
    rank_of[order] = np.arange(n)
    c_of = rank_of % CORES
    j_of = rank_of // CORES
    t_of = j_of // P
    p_of = j_of % P

    # slots: all edges + one self-loop per node, keyed by dst
    slot_nid = np.concatenate([dst, np.arange(n)])
    slot_src = np.concatenate([src, np.arange(n)])

    bases1 = _windows(v1)
    bases2 = _windows(v2)
    # L1 zero rows: natural row 0 is zero; rows n+1..v1-1 are zero.
    zr1 = [0, n + 2] if len(bases1) == 2 else [0]
    assert len(bases1) == 1 or bases1[1] <= n + 2 < v1
    # L2 zero rows: pad nodes j in [npc, jpad) of any core shard are ~0.
    zr2 = [npc]
    if len(bases2) == 2:
        c = next(
            c for c in range(CORES) if bases2[1] <= c * jpad + npc < bases2[1] + WIN
        )
        zr2.append(c * jpad + npc)

    r1 = slot_src + 1  # natural row (+1 for zero row at 0)
    r2 = jpad * c_of[slot_src] + j_of[slot_src]

    J1, Foff1, F1, idx16_1 = _build_layer_tables(
        slot_nid, r1, bases1, zr1, t_of, p_of, c_of, tiles
    )
    J2, Foff2, F2, idx16_2 = _build_layer_tables(
        slot_nid, r2, bases2, zr2, t_of, p_of, c_of, tiles
    )

    # deg per natural row (for the sharded xs pre-pass); pads get 1.0
    degpad = np.ones(v1, dtype=_np_f32)
    degpad[np.arange(n) + 1] = deg
    deg_nat = np.ascontiguousarray(degpad.reshape(v1 // P, P).T)  # [128, v1/128]

    # deg of own (dst) rows, [core][p][tile]; pads get 1e30 -> dinv ~ 0
    deg_own = np.full((CORES, P, tiles), 1e30, dtype=_np_f32)
    jgrid = np.arange(jpad)
    tgrid, pgrid = jgrid // P, jgrid % P
    valid = jgrid < npc
    for c in range(CORES):
        nodes = order[np.minimum(jgrid * CORES + c, n - 1)]
        deg_own[c, pgrid[valid], tgrid[valid]] = deg[nodes[valid]]

    # natural x table rows (zero row 0, rows 1..n = x, rest zero), sharded
    xpad = np.zeros((v1, IN_C), dtype=_np_f32)
    xpad[1 : n + 1] = x

    shapes = dict(
        cfg=cfg,
        bases1=bases1,
        bases2=bases2,
        J1=J1,
        J2=J2,
        Foff1=Foff1,
        Foff2=Foff2,
        F1=F1,
        F2=F2,
    )
    percore = dict(idx1=idx16_1, idx2=idx16_2, deg_own=deg_own)
    return shapes, percore, xpad, deg_nat, order


def _build(shapes):
    from concourse import bass, bacc, mybir, tile
    from concourse.masks import make_identity

    f32 = mybir.dt.float32
    f16 = mybir.dt.float16
    i16 = mybir.dt.int16
    cfg = shapes["cfg"]
    tiles, jpad, v1, v2 = cfg["tiles"], cfg["jpad"], cfg["v1"], cfg["v2"]
    bases1, bases2 = shapes["bases1"], shapes["bases2"]
    J1, J2 = shapes["J1"], shapes["J2"]
    Foff1, Foff2 = shapes["Foff1"], shapes["Foff2"]
    F1, F2 = shapes["F1"], shapes["F2"]
    shard_tiles = v1 // P // CORES  # x row-tiles per core in the pre-pass

    nc = bacc.Bacc(None, target_bir_lowering=False, num_swdge_queues=4)
    xshard = nc.declare_dram_parameter("xshard", [shard_tiles * P, IN_C], f32, isOutput=False)
    w1 = nc.declare_dram_parameter("w1", [IN_C, HID_C], f32, isOutput=False)
    w2 = nc.declare_dram_parameter("w2", [HID_C, OUT_C], f32, isOutput=False)
    b1 = nc.declare_dram_parameter("b1", [P, HID_C], f32, isOutput=False)
    b2 = nc.declare_dram_parameter("b2", [P, OUT_C], f32, isOutput=False)
    idx1 = nc.declare_dram_parameter("idx1", [16, F1], i16, isOutput=False)
    idx2 = nc.declare_dram_parameter("idx2", [16, F2], i16, isOutput=False)
    degsh = nc.declare_dram_parameter("degsh", [P, shard_tiles], f32, isOutput=False)
    deg_own = nc.declare_dram_parameter("deg_own", [P, tiles], f32, isOutput=False)
    zout = nc.declare_dram_parameter("zout", [jpad, OUT_C], f32, isOutput=True)

    mult = mybir.AluOpType.mult
    qctr = [0]

    def bcast_last(ap, cnt):
        return bass.AP(ap.tensor, ap.offset, list(ap.ap) + [[0, cnt]])

    def emit_gathers(g_t, table_handle, base, idx_t, Jtw, fo, joff):
        j0 = 0
        while j0 < Jtw:
            jc = min(MAX_J_CHUNK, Jtw - j0)
            nc.gpsimd.dma_gather(
                out_ap=g_t[:, joff + j0 : joff + j0 + jc, :],
                in_ap=table_handle[base:, :],
                idxs_ap=idx_t[:, fo + 8 * j0 : fo + 8 * (j0 + jc)],
                num_idxs=P * jc,
                num_idxs_reg=P * jc,
                elem_size=P,
                single_packet=False,
                queue_num=qctr[0] % 4,
            )
            qctr[0] += 1
            j0 += jc

    def tree_reduce64(g_t, Jtot, agg_f32):
        """Fold slot dim (payload cols 0:64) down to 2 in f16, then one
        mixed add into the f32 agg tile."""
        J = Jtot
        while J > 2:
            a = J // 2
            nc.vector.tensor_add(
                out=g_t[:, 0:a, 0:HID_C],
                in0=g_t[:, 0:a, 0:HID_C],
                in1=g_t[:, J - a : J, 0:HID_C],
            )
            J -= a
        if J == 2:
            nc.vector.tensor_add(
                out=agg_f32[:], in0=g_t[:, 0, 0:HID_C], in1=g_t[:, 1, 0:HID_C]
            )
        else:
            nc.vector.tensor_copy(agg_f32[:], g_t[:, 0, 0:HID_C])

    with tile.TileContext(nc) as tc:
        with (
            tc.tile_pool(name="const", bufs=1) as cp,
            tc.tile_pool(name="dram", bufs=1, space="DRAM") as dp,
        ):
            w1_t = cp.tile([IN_C, HID_C], f32)
            nc.sync.dma_start(w1_t[:], w1[:])
            w2_t = cp.tile([HID_C, OUT_C], f32)
            nc.sync.dma_start(w2_t[:], w2[:])
            b1_t = cp.tile([P, HID_C], f32)
            nc.sync.dma_start(b1_t[:], b1[:])
            b2_t = cp.tile([P, OUT_C], f32)
            nc.sync.dma_start(b2_t[:], b2[:])

            ident_t = cp.tile([P, P], f32)
            make_identity(nc, ident_t[:])

            deg_t = cp.tile([P, tiles], f32)
            nc.sync.dma_start(deg_t[:], deg_own[:])
            dinv_t = cp.tile([P, tiles], f32)
            nc.vector.reciprocal(dinv_t[:], deg_t[:])
            nc.scalar.activation(
                dinv_t[:], dinv_t[:], mybir.ActivationFunctionType.Sqrt
            )

            degn_t = cp.tile([P, shard_tiles], f32)
            nc.sync.dma_start(degn_t[:], degsh[:])
            dinvn_t = cp.tile([P, shard_tiles], f32)
            nc.vector.reciprocal(dinvn_t[:], degn_t[:])
            nc.scalar.activation(
                dinvn_t[:], dinvn_t[:], mybir.ActivationFunctionType.Sqrt
            )

            xs_shard = dp.tile([shard_tiles * P, P], f16)  # cols 0:64 payload
            xs_full = dp.tile([v1, P], f16)
            u2shard = dp.tile([jpad, P], f16)
            table2 = dp.tile([v2, P], f16)

            # ---- pre-pass: xs_shard = f16(dinv ⊙ (x @ W1)) for our rows ----
            with (
                tc.tile_pool(name="pre", bufs=3) as prep_pool,
                tc.tile_pool(name="preps", bufs=2, space="PSUM") as ppp,
            ):
                for k in range(shard_tiles):
                    xt = prep_pool.tile([P, IN_C], f32, tag="xt")
                    nc.sync.dma_start(xt[:], xshard[P * k : P * (k + 1), :])
                    psT = ppp.tile([IN_C, P], f32, tag="psT")
                    nc.tensor.transpose(psT[:], xt[:], ident_t[:])
                    xT = prep_pool.tile([IN_C, P], f32, tag="xT")
                    nc.vector.tensor_copy(xT[:], psT[:])
                    ps1 = ppp.tile([P, HID_C], f32, tag="ps1")
                    nc.tensor.matmul(
                        out=ps1[:], lhsT=xT[:], rhs=w1_t[:], start=True, stop=True
                    )
                    xsb = prep_pool.tile([P, HID_C], f16, tag="xsb")
                    nc.vector.tensor_tensor(
                        out=xsb[:],
                        in0=ps1[:],
                        in1=bcast_last(dinvn_t[:, k : k + 1], HID_C),
                        op=mult,
                    )
                    nc.sync.dma_start(
                        xs_shard[P * k : P * (k + 1), 0:HID_C], xsb[:]
                    )

            # ---- all-gather xs shards -> full L1 table ----
            nc.gpsimd.collective_compute(
                "AllGather",
                mybir.AluOpType.bypass,
                replica_groups=[list(range(CORES))],
                ins=[xs_shard[:]],
                outs=[xs_full[:]],
            )

            # ---- layer 1 ----
            with (
                tc.tile_pool(name="idx1p", bufs=1) as ip1,
                tc.tile_pool(name="g1", bufs=3) as gp1,
                tc.tile_pool(name="l1s", bufs=2) as sp1,
                tc.tile_pool(name="ps", bufs=2, space="PSUM") as pp,
            ):
                idx1_t = ip1.tile([P, F1], i16)
                for g in range(8):
                    nc.sync.dma_start(idx1_t[16 * g : 16 * (g + 1), :], idx1[:, :])

                for t in range(tiles):
                    Jtot = int(J1[t].sum())
                    g_t = gp1.tile([P, Jtot, P], f16, tag="g1")
                    joff = 0
                    for w in range(len(bases1)):
                        if J1[t, w] == 0:
                            continue
                        emit_gathers(
                            g_t, xs_full, bases1[w], idx1_t,
                            int(J1[t, w]), int(Foff1[t, w]), joff,
                        )
                        joff += int(J1[t, w])
                    agg_t = sp1.tile([P, HID_C], f32, tag="agg")
                    tree_reduce64(g_t, Jtot, agg_t)
                    h = sp1.tile([P, HID_C], f32, tag="h")
                    nc.vector.tensor_tensor(
                        out=h[:],
                        in0=agg_t[:],
                        in1=bcast_last(dinv_t[:, t : t + 1], HID_C),
                        op=mult,
                    )
                    nc.vector.tensor_add(out=h[:], in0=h[:], in1=b1_t[:])
                    nc.scalar.activation(h[:], h[:], mybir.ActivationFunctionType.Relu)
                    psT = pp.tile([HID_C, P], f32, tag="psT")
                    nc.tensor.transpose(psT[:], h[:], ident_t[:])
                    hT = sp1.tile([HID_C, P], f32, tag="hT")
                    nc.vector.tensor_copy(hT[:], psT[:])
                    ps2 = pp.tile([P, OUT_C], f32, tag="ps2")
                    nc.tensor.matmul(
                        out=ps2[:], lhsT=hT[:], rhs=w2_t[:], start=True, stop=True
                    )
                    u2 = sp1.tile([P, OUT_C], f16, tag="u2")
                    nc.vector.tensor_tensor(
                        out=u2[:],
                        in0=ps2[:],
                        in1=bcast_last(dinv_t[:, t : t + 1], OUT_C),
                        op=mult,
                    )
                    nc.sync.dma_start(u2shard[P * t : P * (t + 1), 0:OUT_C], u2[:])

            # ---- all-gather u2 shards -> full L2 table ----
            nc.gpsimd.collective_compute(
                "AllGather",
                mybir.AluOpType.bypass,
                replica_groups=[list(range(CORES))],
                ins=[u2shard[:]],
                outs=[table2[:]],
            )

            # ---- layer 2 ----
            with (
                tc.tile_pool(name="idx2p", bufs=1) as ip2,
                tc.tile_pool(name="g2", bufs=3) as gp2,
                tc.tile_pool(name="l2s", bufs=2) as sp2,
            ):
                idx2_t = ip2.tile([P, F2], i16)
                for g in range(8):
                    nc.sync.dma_start(idx2_t[16 * g : 16 * (g + 1), :], idx2[:, :])

                for t in range(tiles):
                    Jtot = int(J2[t].sum())
                    g_t = gp2.tile([P, Jtot, P], f16, tag="g2")
                    joff = 0
                    for w in range(len(bases2)):
                        if J2[t, w] == 0:
                            continue
                        emit_gathers(
                            g_t, table2, bases2[w], idx2_t,
                            int(J2[t, w]), int(Foff2[t, w]), joff,
                        )
                        joff += int(J2[t, w])
                    agg2_t = sp2.tile([P, OUT_C], f32, tag="agg2")
                    tree_reduce64(g_t, Jtot, agg2_t)
                    z = sp2.tile([P, OUT_C], f32, tag="z")
                    nc.vector.tensor_tensor(
                        out=z[:],
                        in0=agg2_t[:],
                        in1=bcast_last(dinv_t[:, t : t + 1], OUT_C),
                        op=mult,
                    )
                    nc.vector.tensor_add(out=z[:], in0=z[:], in1=b2_t[:])
                    nc.sync.dma_start(zout[P * t : P * (t + 1), :], z[:])

    nc.finalize()
    return nc


def kernel(x, edge_index, W1, b1, W2, b2):
    from concourse.bass_utils import run_bass_kernel_spmd

    x = np.ascontiguousarray(np.asarray(x, dtype=_np_f32))
    n = x.shape[0]
    shapes, percore, xpad, deg_nat, order = _prep(x, edge_index)
    nc = _build(shapes)

    cfg = shapes["cfg"]
    shard_rows = cfg["v1"] // CORES
    shard_tiles = shard_rows // P

    b1_bc = np.ascontiguousarray(np.broadcast_to(np.asarray(b1, _np_f32), (P, HID_C)))
    b2_bc = np.ascontiguousarray(np.broadcast_to(np.asarray(b2, _np_f32), (P, OUT_C)))
    W1a = np.ascontiguousarray(np.asarray(W1, _np_f32))
    W2a = np.ascontiguousarray(np.asarray(W2, _np_f32))

    in_maps = []
    for c in range(CORES):
        in_maps.append(
            {
                "xshard": np.ascontiguousarray(
                    xpad[c * shard_rows : (c + 1) * shard_rows]
                ),
                "w1": W1a,
                "w2": W2a,
                "b1": b1_bc,
                "b2": b2_bc,
                "idx1": np.ascontiguousarray(percore["idx1"][c]),
                "idx2": np.ascontiguousarray(percore["idx2"][c]),
                "degsh": np.ascontiguousarray(
                    deg_nat[:, c * shard_tiles : (c + 1) * shard_tiles]
                ),
                "deg_own": np.ascontiguousarray(percore["deg_own"][c]),
            }
        )

    res = run_bass_kernel_spmd(nc, in_maps, list(range(CORES)))

    npc = cfg["npc"]
    z = np.empty((n, OUT_C), dtype=_np_f32)
    for c in range(CORES):
        zc = res.results[c]["zout"][:npc]
        nodes = order[np.arange(npc) * CORES + c]
        valid = np.arange(npc) * CORES + c < n
        z[nodes[valid]] = zc[valid]
    return z


# revision 11
# speedup vs baseline: 1.1248x; 1.1248x over previous
"""2-layer GCN on 8 Trainium2 NeuronCores — v2 (transform-first ELL gather).

Strategy:
  - Nodes ranked by in-degree (desc), dealt round-robin to 8 cores; core c
    owns dst nodes rank%8==c. Per-core ELL: tile t covers 128 dst nodes,
    J[t] slot-columns; gathers via gpsimd.dma_gather (int16 idx).
  - GCN norm factorizes: out = dinv ⊙ ((A+I)(dinv ⊙ h)); aggregation is an
    unweighted gather+sum of pre-scaled rows.
  - BOTH layers are transform-first: the gather tables hold 64-wide
    dinv-scaled transformed features in f16, stored as [rows, 128] with
    cols 64:128 unused junk (gather elements must be 256B multiples).
    Tree-reduce touches only cols 0:64 (strided views).
  - L1 table xs = dinv ⊙ (x @ W1): computed sharded (1/8 rows per core),
    assembled with AllGather. L2 table u2 = dinv ⊙ (relu(h1) @ W2): per-core
    shard + AllGather (as baseline).
  - int16 idx range handled with 2 overlapping 32768-row windows
    (bases 0 and v-32768); middle rows balanced per dst node.
"""

import sys

sys.path.insert(0, "/opt/trn_rl_repo")

import numpy as np

P = 128
CORES = 8
WIN = 32768
IN_C = 128
HID_C = 64
OUT_C = 64
MAX_J_CHUNK = 64  # 128*64 = 8192 idx / dma_gather instruction

_np_f32 = np.float32


def _cfg(n):
    npc = -(-n // CORES)
    tiles = -(-npc // P)
    jpad = tiles * P
    v1 = -(-(n + 1) // (P * CORES)) * P * CORES  # natural table rows (row0=zero)
    v2 = CORES * jpad
    return dict(n=n, npc=npc, tiles=tiles, jpad=jpad, v1=v1, v2=v2)


def _windows(v):
    if v <= WIN:
        return [0]
    return [0, (v - WIN) // 2, v - WIN]


def _cumcount(key):
    """Occurrence index of each element within its key-group (stable)."""
    order = np.argsort(key, kind="stable")
    ks = key[order]
    grp_start = np.r_[0, np.flatnonzero(np.diff(ks)) + 1]
    seg = np.searchsorted(grp_start, np.arange(len(ks)), side="right") - 1
    q = np.arange(len(ks)) - grp_start[seg]
    out = np.empty(len(ks), dtype=np.int64)
    out[order] = q
    return out


def _assign_windows(nid, r, bases):
    """Assign each slot to one feasible window, balancing per-node counts."""
    nwin = len(bases)
    if nwin == 1:
        return np.zeros(len(nid), dtype=np.int64)
    b0, bm, bb = bases
    in_a = r < WIN
    in_m = (r >= bm) & (r < bm + WIN)
    in_b = r >= bb
    cls = in_a * 1 + in_m * 2 + in_b * 4  # 1,3,7,6,4
    nmax = int(nid.max()) + 1
    cnt = np.bincount(nid * 8 + cls, minlength=nmax * 8).reshape(nmax, 8)
    n0, n1, n2, n3, n4 = cnt[:, 1], cnt[:, 3], cnt[:, 7], cnt[:, 6], cnt[:, 4]
    K = n0 + n1 + n2 + n3 + n4
    ideal = (K + 2) // 3
    t1 = np.clip(ideal - n0, 0, n1)
    aA = n0 + t1
    t3 = np.clip(ideal - n4, 0, n3)
    aB = n4 + t3
    addA = np.clip(ideal - aA, 0, n2)
    rem = n2 - addA
    addB = np.clip(ideal - aB, 0, rem)

    q = _cumcount(nid * 8 + cls)
    win = np.full(len(nid), 1, dtype=np.int64)
    win[cls == 1] = 0
    win[cls == 4] = 2
    m = cls == 3
    win[m] = np.where(q[m] < t1[nid[m]], 0, 1)
    m = cls == 6
    win[m] = np.where(q[m] < t3[nid[m]], 2, 1)
    m = cls == 7
    qa = q[m]
    na = nid[m]
    win[m] = np.where(qa < addA[na], 0, np.where(qa < addA[na] + addB[na], 2, 1))
    ok = (r >= np.array(bases)[win]) & (r < np.array(bases)[win] + WIN)
    assert ok.all(), "window assignment out of range"
    return win


def _build_layer_tables(slot_nid, slot_r, bases, zero_rows, t_of, p_of, c_of, tiles):
    """Per-core int16 idx arrays + shape info for one layer.

    Returns (J [tiles,nwin], Foff [tiles,nwin], F, idx16 [CORES,16,F])."""
    nwin = len(bases)
    win = _assign_windows(slot_nid, slot_r, bases)
    jslot = _cumcount(slot_nid * 4 + win)

    t_slot = t_of[slot_nid]
    J = np.zeros((tiles, nwin), dtype=np.int64)
    np.maximum.at(J, (t_slot, win), jslot + 1)

    blk = 8 * J  # idx cols per (tile, window) block
    Foff = np.zeros((tiles, nwin), dtype=np.int64)
    flat = blk.reshape(-1)
    Foff.reshape(-1)[1:] = np.cumsum(flat)[:-1]
    F = int(flat.sum())

    idx16 = np.empty((CORES, 16, F), dtype=np.int16)
    for t in range(tiles):
        for w in range(nwin):
            if J[t, w] == 0:
                continue
            fo, ln = Foff[t, w], blk[t, w]
            idx16[:, :, fo : fo + ln] = np.int16(zero_rows[w] - bases[w])
    p_slot = p_of[slot_nid]
    c_slot = c_of[slot_nid]
    i = jslot * P + p_slot
    col = Foff[t_slot, win] + i // 16
    row = i % 16
    val = (slot_r - np.array(bases)[win]).astype(np.int16)
    idx16[c_slot, row, col] = val
    return J, Foff, F, idx16


def _prep(x, edge_index):
    n = x.shape[0]
    cfg = _cfg(n)
    npc, tiles, jpad, v1, v2 = (
        cfg["npc"],
        cfg["tiles"],
        cfg["jpad"],
        cfg["v1"],
        cfg["v2"],
    )
    assert npc < jpad, "shard needs pad rows (zero-gather targets)"
    assert v1 % (P * CORES) == 0
    src = np.asarray(edge_index[0], dtype=np.int64)
    dst = np.asarray(edge_index[1], dtype=np.int64)
    deg = np.bincount(dst, minlength=n) + 1  # in-degree + self-loop

    order = np.argsort(-deg, kind="stable")  # rank -> node
    rank_of = np.empty(n, dtype=np.int64)
    rank_of[order] = np.arange(n)
    c_of = rank_of % CORES
    j_of = rank_of // CORES
    t_of = j_of // P
    p_of = j_of % P

    # slots: all edges + one self-loop per node, keyed by dst
    slot_nid = np.concatenate([dst, np.arange(n)])
    slot_src = np.concatenate([src, np.arange(n)])

    bases1 = _windows(v1)
    bases2 = _windows(v2)
    n_split = n // 2
    # L1 zero rows per window (xpad zeros: row 0, row n_split+1, rows n+2..)
    zr1 = []
    for b in bases1:
        cands = [0, n_split + 1, n + 2]
        zr1.append(next(c for c in cands if b <= c < b + WIN))
    # L2 table row layout: two chunked AllGathers -> [8 x rows_a | 8 x rows_b]
    tiles_a = (tiles + 1) // 2
    rows_a = tiles_a * P
    rows_b = jpad - rows_a
    assert npc >= rows_a, "pad (zero) u2 rows must land in the second chunk"
    # L2 zero rows: pad nodes j in [npc, jpad) of any core shard are ~0.
    zr2 = []
    for b in bases2:
        zr2.append(
            next(
                CORES * rows_a + c * rows_b + (npc - rows_a)
                for c in range(CORES)
                if b <= CORES * rows_a + c * rows_b + (npc - rows_a) < b + WIN
            )
        )

    # natural row with zero rows at 0 and n_split+1
    r1 = np.where(slot_src < n_split, slot_src + 1, slot_src + 2)
    j_src = j_of[slot_src]
    c_src = c_of[slot_src]
    r2 = np.where(
        j_src < rows_a,
        rows_a * c_src + j_src,
        CORES * rows_a + rows_b * c_src + (j_src - rows_a),
    )

    J1, Foff1, F1, idx16_1 = _build_layer_tables(
        slot_nid, r1, bases1, zr1, t_of, p_of, c_of, tiles
    )
    J2, Foff2, F2, idx16_2 = _build_layer_tables(
        slot_nid, r2, bases2, zr2, t_of, p_of, c_of, tiles
    )

    # deg per natural row (for the sharded xs pre-pass); pads get 1.0
    nat_rows = np.where(np.arange(n) < n_split, np.arange(n) + 1, np.arange(n) + 2)
    degpad = np.ones(v1, dtype=_np_f32)
    degpad[nat_rows] = deg
    deg_nat = np.ascontiguousarray(degpad.reshape(v1 // P, P).T)  # [128, v1/128]

    # deg of own (dst) rows, [core][p][tile]; pads get 1e30 -> dinv ~ 0
    deg_own = np.full((CORES, P, tiles), 1e30, dtype=_np_f32)
    jgrid = np.arange(jpad)
    tgrid, pgrid = jgrid // P, jgrid % P
    valid = jgrid < npc
    for c in range(CORES):
        nodes = order[np.minimum(jgrid * CORES + c, n - 1)]
        deg_own[c, pgrid[valid], tgrid[valid]] = deg[nodes[valid]]

    # natural x table rows (zero rows at 0 and n_split+1, rest zero), sharded
    xpad = np.zeros((v1, IN_C), dtype=_np_f32)
    xpad[nat_rows] = x

    shapes = dict(
        cfg=cfg,
        bases1=bases1,
        bases2=bases2,
        J1=J1,
        J2=J2,
        Foff1=Foff1,
        Foff2=Foff2,
        F1=F1,
        F2=F2,
    )
    percore = dict(idx1=idx16_1, idx2=idx16_2, deg_own=deg_own)
    return shapes, percore, xpad, deg_nat, order


def _build(shapes):
    from concourse import bass, bacc, mybir, tile
    from concourse.masks import make_identity

    f32 = mybir.dt.float32
    f16 = mybir.dt.float16
    i16 = mybir.dt.int16
    cfg = shapes["cfg"]
    tiles, jpad, v1, v2 = cfg["tiles"], cfg["jpad"], cfg["v1"], cfg["v2"]
    bases1, bases2 = shapes["bases1"], shapes["bases2"]
    J1, J2 = shapes["J1"], shapes["J2"]
    Foff1, Foff2 = shapes["Foff1"], shapes["Foff2"]
    F1, F2 = shapes["F1"], shapes["F2"]
    shard_tiles = v1 // P // CORES  # x row-tiles per core in the pre-pass

    nc = bacc.Bacc(None, target_bir_lowering=False, num_swdge_queues=4)
    xshard = nc.declare_dram_parameter("xshard", [shard_tiles * P, IN_C], f32, isOutput=False)
    w1 = nc.declare_dram_parameter("w1", [IN_C, HID_C], f32, isOutput=False)
    w2 = nc.declare_dram_parameter("w2", [HID_C, OUT_C], f32, isOutput=False)
    b1 = nc.declare_dram_parameter("b1", [P, HID_C], f32, isOutput=False)
    b2 = nc.declare_dram_parameter("b2", [P, OUT_C], f32, isOutput=False)
    idx1 = nc.declare_dram_parameter("idx1", [16, F1], i16, isOutput=False)
    idx2 = nc.declare_dram_parameter("idx2", [16, F2], i16, isOutput=False)
    degsh = nc.declare_dram_parameter("degsh", [P, shard_tiles], f32, isOutput=False)
    deg_own = nc.declare_dram_parameter("deg_own", [P, tiles], f32, isOutput=False)
    zout = nc.declare_dram_parameter("zout", [jpad, OUT_C], f32, isOutput=True)

    mult = mybir.AluOpType.mult
    qctr = [0]

    def bcast_last(ap, cnt):
        return bass.AP(ap.tensor, ap.offset, list(ap.ap) + [[0, cnt]])

    def emit_gathers(g_t, table_handle, base, idx_t, Jtw, fo, joff):
        j0 = 0
        while j0 < Jtw:
            jc = min(MAX_J_CHUNK, Jtw - j0)
            nc.gpsimd.dma_gather(
                out_ap=g_t[:, joff + j0 : joff + j0 + jc, :],
                in_ap=table_handle[base:, :],
                idxs_ap=idx_t[:, fo + 8 * j0 : fo + 8 * (j0 + jc)],
                num_idxs=P * jc,
                num_idxs_reg=P * jc,
                elem_size=P,
                single_packet=False,
                queue_num=qctr[0] % 4,
            )
            qctr[0] += 1
            j0 += jc

    def tree_reduce64(g_t, Jtot, agg_f32):
        """Fold slot dim (payload cols 0:64) down to 2 in f16, then one
        mixed add into the f32 agg tile."""
        J = Jtot
        while J > 2:
            a = J // 2
            nc.vector.tensor_add(
                out=g_t[:, 0:a, 0:HID_C],
                in0=g_t[:, 0:a, 0:HID_C],
                in1=g_t[:, J - a : J, 0:HID_C],
            )
            J -= a
        if J == 2:
            nc.vector.tensor_add(
                out=agg_f32[:], in0=g_t[:, 0, 0:HID_C], in1=g_t[:, 1, 0:HID_C]
            )
        else:
            nc.vector.tensor_copy(agg_f32[:], g_t[:, 0, 0:HID_C])

    with tile.TileContext(nc) as tc:
        with (
            tc.tile_pool(name="const", bufs=1) as cp,
            tc.tile_pool(name="dram", bufs=1, space="DRAM") as dp,
        ):
            w1_t = cp.tile([IN_C, HID_C], f32)
            nc.sync.dma_start(w1_t[:], w1[:])
            w2_t = cp.tile([HID_C, OUT_C], f32)
            nc.sync.dma_start(w2_t[:], w2[:])
            b1_t = cp.tile([P, HID_C], f32)
            nc.sync.dma_start(b1_t[:], b1[:])
            b2_t = cp.tile([P, OUT_C], f32)
            nc.sync.dma_start(b2_t[:], b2[:])

            ident_t = cp.tile([P, P], f32)
            make_identity(nc, ident_t[:])

            deg_t = cp.tile([P, tiles], f32)
            nc.sync.dma_start(deg_t[:], deg_own[:])
            dinv_t = cp.tile([P, tiles], f32)
            nc.vector.reciprocal(dinv_t[:], deg_t[:])
            nc.scalar.activation(
                dinv_t[:], dinv_t[:], mybir.ActivationFunctionType.Sqrt
            )

            degn_t = cp.tile([P, shard_tiles], f32)
            nc.sync.dma_start(degn_t[:], degsh[:])
            dinvn_t = cp.tile([P, shard_tiles], f32)
            nc.vector.reciprocal(dinvn_t[:], degn_t[:])
            nc.scalar.activation(
                dinvn_t[:], dinvn_t[:], mybir.ActivationFunctionType.Sqrt
            )

            tiles_a = (tiles + 1) // 2
            rows_a = tiles_a * P  # first-chunk rows per core shard
            xs_shard = dp.tile([shard_tiles * P, P], f16)  # cols 0:64 payload
            xs_full = dp.tile([v1, P], f16)
            u2shard_a = dp.tile([rows_a, P], f16)
            u2shard_b = dp.tile([jpad - rows_a, P], f16)
            table2 = dp.tile([v2, P], f16)

            # ---- pre-pass: xs_shard = f16(dinv ⊙ (x @ W1)) for our rows ----
            with (
                tc.tile_pool(name="pre", bufs=3) as prep_pool,
                tc.tile_pool(name="preps", bufs=2, space="PSUM") as ppp,
            ):
                for k in range(shard_tiles):
                    xt = prep_pool.tile([P, IN_C], f32, tag="xt")
                    nc.sync.dma_start(xt[:], xshard[P * k : P * (k + 1), :])
                    psT = ppp.tile([IN_C, P], f32, tag="psT")
                    nc.tensor.transpose(psT[:], xt[:], ident_t[:])
                    xT = prep_pool.tile([IN_C, P], f32, tag="xT")
                    nc.vector.tensor_copy(xT[:], psT[:])
                    ps1 = ppp.tile([P, HID_C], f32, tag="ps1")
                    nc.tensor.matmul(
                        out=ps1[:], lhsT=xT[:], rhs=w1_t[:], start=True, stop=True
                    )
                    xsb = prep_pool.tile([P, HID_C], f16, tag="xsb")
                    nc.vector.tensor_tensor(
                        out=xsb[:],
                        in0=ps1[:],
                        in1=bcast_last(dinvn_t[:, k : k + 1], HID_C),
                        op=mult,
                    )
                    nc.sync.dma_start(
                        xs_shard[P * k : P * (k + 1), 0:HID_C], xsb[:]
                    )

            # ---- all-gather xs shards -> full L1 table ----
            nc.gpsimd.collective_compute(
                "AllGather",
                mybir.AluOpType.bypass,
                replica_groups=[list(range(CORES))],
                ins=[xs_shard[:]],
                outs=[xs_full[:]],
            )

            # ---- layers (idx pools span both; idx2 load overlaps L1) ----
            with tc.tile_pool(name="idxp", bufs=1) as ip1:
                idx1_t = ip1.tile([P, F1], i16)
                for g in range(8):
                    nc.sync.dma_start(idx1_t[16 * g : 16 * (g + 1), :], idx1[:, :])
                idx2_t = ip1.tile([P, F2], i16)
                for g in range(8):
                    nc.sync.dma_start(idx2_t[16 * g : 16 * (g + 1), :], idx2[:, :])

                l1_pools = (
                    tc.tile_pool(name="g1", bufs=6),
                    tc.tile_pool(name="l1s", bufs=4),
                    tc.tile_pool(name="ps", bufs=4, space="PSUM"),
                )
                gp1, sp1, pp = [p.__enter__() for p in l1_pools]

                for t in range(tiles):
                    if t == tiles_a:
                        # first-half u2 shard complete: overlap its AllGather
                        nc.gpsimd.collective_compute(
                            "AllGather",
                            mybir.AluOpType.bypass,
                            replica_groups=[list(range(CORES))],
                            ins=[u2shard_a[:]],
                            outs=[table2[0 : CORES * rows_a, :]],
                        )
                    Jtot = int(J1[t].sum())
                    g_t = gp1.tile([P, Jtot, P], f16, tag="g1")
                    joff = 0
                    for w in range(len(bases1)):
                        if J1[t, w] == 0:
                            continue
                        emit_gathers(
                            g_t, xs_full, bases1[w], idx1_t,
                            int(J1[t, w]), int(Foff1[t, w]), joff,
                        )
                        joff += int(J1[t, w])
                    agg_t = sp1.tile([P, HID_C], f32, tag="agg")
                    tree_reduce64(g_t, Jtot, agg_t)
                    h = sp1.tile([P, HID_C], f32, tag="h")
                    nc.vector.tensor_tensor(
                        out=h[:],
                        in0=agg_t[:],
                        in1=bcast_last(dinv_t[:, t : t + 1], HID_C),
                        op=mult,
                    )
                    nc.vector.tensor_add(out=h[:], in0=h[:], in1=b1_t[:])
                    nc.scalar.activation(h[:], h[:], mybir.ActivationFunctionType.Relu)
                    psT = pp.tile([HID_C, P], f32, tag="psT")
                    nc.tensor.transpose(psT[:], h[:], ident_t[:])
                    hT = sp1.tile([HID_C, P], f32, tag="hT")
                    nc.vector.tensor_copy(hT[:], psT[:])
                    ps2 = pp.tile([P, OUT_C], f32, tag="ps2")
                    nc.tensor.matmul(
                        out=ps2[:], lhsT=hT[:], rhs=w2_t[:], start=True, stop=True
                    )
                    u2 = sp1.tile([P, OUT_C], f16, tag="u2")
                    nc.vector.tensor_tensor(
                        out=u2[:],
                        in0=ps2[:],
                        in1=bcast_last(dinv_t[:, t : t + 1], OUT_C),
                        op=mult,
                    )
                    if t < tiles_a:
                        nc.sync.dma_start(
                            u2shard_a[P * t : P * (t + 1), 0:OUT_C], u2[:]
                        )
                    else:
                        tb = t - tiles_a
                        nc.sync.dma_start(
                            u2shard_b[P * tb : P * (tb + 1), 0:OUT_C], u2[:]
                        )

                for p in reversed(l1_pools):
                    p.__exit__(None, None, None)

                # ---- all-gather second-half u2 shards ----
                nc.gpsimd.collective_compute(
                    "AllGather",
                    mybir.AluOpType.bypass,
                    replica_groups=[list(range(CORES))],
                    ins=[u2shard_b[:]],
                    outs=[table2[CORES * rows_a :, :]],
                )

                # ---- layer 2 ----
                with (
                    tc.tile_pool(name="g2", bufs=6) as gp2,
                    tc.tile_pool(name="l2s", bufs=4) as sp2,
                ):
                  for t in range(tiles):
                    Jtot = int(J2[t].sum())
                    g_t = gp2.tile([P, Jtot, P], f16, tag="g2")
                    joff = 0
                    for w in range(len(bases2)):
                        if J2[t, w] == 0:
                            continue
                        emit_gathers(
                            g_t, table2, bases2[w], idx2_t,
                            int(J2[t, w]), int(Foff2[t, w]), joff,
                        )
                        joff += int(J2[t, w])
                    agg2_t = sp2.tile([P, OUT_C], f32, tag="agg2")
                    tree_reduce64(g_t, Jtot, agg2_t)
                    z = sp2.tile([P, OUT_C], f32, tag="z")
                    nc.vector.tensor_tensor(
                        out=z[:],
                        in0=agg2_t[:],
                        in1=bcast_last(dinv_t[:, t : t + 1], OUT_C),
                        op=mult,
                    )
                    nc.vector.tensor_add(out=z[:], in0=z[:], in1=b2_t[:])
                    nc.sync.dma_start(zout[P * t : P * (t + 1), :], z[:])

    nc.finalize()
    return nc


def kernel(x, edge_index, W1, b1, W2, b2):
    from concourse.bass_utils import run_bass_kernel_spmd

    x = np.ascontiguousarray(np.asarray(x, dtype=_np_f32))
    n = x.shape[0]
    shapes, percore, xpad, deg_nat, order = _prep(x, edge_index)
    nc = _build(shapes)

    cfg = shapes["cfg"]
    shard_rows = cfg["v1"] // CORES
    shard_tiles = shard_rows // P

    b1_bc = np.ascontiguousarray(np.broadcast_to(np.asarray(b1, _np_f32), (P, HID_C)))
    b2_bc = np.ascontiguousarray(np.broadcast_to(np.asarray(b2, _np_f32), (P, OUT_C)))
    W1a = np.ascontiguousarray(np.asarray(W1, _np_f32))
    W2a = np.ascontiguousarray(np.asarray(W2, _np_f32))

    in_maps = []
    for c in range(CORES):
        in_maps.append(
            {
                "xshard": np.ascontiguousarray(
                    xpad[c * shard_rows : (c + 1) * shard_rows]
                ),
                "w1": W1a,
                "w2": W2a,
                "b1": b1_bc,
                "b2": b2_bc,
                "idx1": np.ascontiguousarray(percore["idx1"][c]),
                "idx2": np.ascontiguousarray(percore["idx2"][c]),
                "degsh": np.ascontiguousarray(
                    deg_nat[:, c * shard_tiles : (c + 1) * shard_tiles]
                ),
                "deg_own": np.ascontiguousarray(percore["deg_own"][c]),
            }
        )

    res = run_bass_kernel_spmd(nc, in_maps, list(range(CORES)))

    npc = cfg["npc"]
    z = np.empty((n, OUT_C), dtype=_np_f32)
    for c in range(CORES):
        zc = res.results[c]["zout"][:npc]
        nodes = order[np.arange(npc) * CORES + c]
        valid = np.arange(npc) * CORES + c < n
        z[nodes[valid]] = zc[valid]
    return z


# revision 13
# speedup vs baseline: 1.3859x; 1.2321x over previous
"""2-layer GCN on 8 Trainium2 NeuronCores.

Strategy (dst-sharded ELL gather):
  - Nodes are ranked by in-degree (desc) and dealt round-robin to the 8
    cores, so every core/tile sees a near-identical degree profile and
    per-tile ELL padding stays small.
  - GCN norm factorizes: out = dinv ⊙ ((A+I) (dinv ⊙ h)), so the
    aggregation is an unweighted row-gather + sum; the per-edge norm
    becomes a per-source scale.
  - Layer 1 aggregates raw x rows (512B gathers) weighted by
    rsqrt(deg[src]) computed on device; then matmul W1, scale, bias, relu.
  - Layer 2 is transform-first: u2 = dinv ⊙ (h1 @ W2) per core, an
    AllGather builds the full [50176, 64] table, then 256B gathers + sum.
  - Gathers use dma_gather (int16 indices). The int16 range limit is
    handled with up to three overlapping 32768-row windows of the table;
    each slot is assigned to a feasible window, balancing per-partition
    slot counts.

The host side only does integer index bookkeeping (sort/bincount/ELL
construction); all floating-point math runs on the NeuronCores.
"""

import sys

sys.path.insert(0, "/opt/trn_rl_repo")

import numpy as np

P = 128
CORES = 8
WIN = 32768
IN_C = 128
HID_C = 64
OUT_C = 64
MAX_J_CHUNK = 64  # 128*64 = 8192 idx / dma_gather instruction

_np_f32 = np.float32


def _cfg(n):
    npc = -(-n // CORES)
    tiles = -(-npc // P)
    jpad = tiles * P
    v2 = CORES * jpad
    n_split = n // 2
    v1 = -(-(n + 3) // P) * P
    return dict(n=n, npc=npc, tiles=tiles, jpad=jpad, v1=v1, v2=v2, n_split=n_split)


def _windows(v):
    if v <= WIN:
        return [0]
    return [0, (v - WIN) // 2, v - WIN]


def _zero_rows_l1(cfg, bases):
    # xpad layout: [0]=0, [1..n_split]=x[:n_split], [n_split+1]=0,
    # [n_split+2 .. n+1]=x[n_split:], [n+2..v1) = 0
    zr = []
    for b in bases:
        cands = [0, cfg["n_split"] + 1, cfg["n"] + 2]
        z = next(c for c in cands if b <= c < b + WIN)
        zr.append(z)
    return zr


def _zero_rows_l2(cfg, bases):
    zr = []
    for b in bases:
        z = next(
            c * cfg["jpad"] + cfg["npc"]
            for c in range(CORES)
            if b <= c * cfg["jpad"] + cfg["npc"] < b + WIN
        )
        zr.append(z)
    return zr


def _row1(src, cfg):
    return np.where(src < cfg["n_split"], src + 1, src + 2).astype(np.int64)


def _cumcount(key):
    """Occurrence index of each element within its key-group (stable)."""
    order = np.argsort(key, kind="stable")
    ks = key[order]
    grp_start = np.r_[0, np.flatnonzero(np.diff(ks)) + 1]
    seg = np.searchsorted(grp_start, np.arange(len(ks)), side="right") - 1
    q = np.arange(len(ks)) - grp_start[seg]
    out = np.empty(len(ks), dtype=np.int64)
    out[order] = q
    return out


def _assign_windows(nid, r, deg, bases):
    """Assign each slot to one feasible window, balancing per-node counts.

    Returns win (0/1/2 per slot)."""
    nwin = len(bases)
    if nwin == 1:
        return np.zeros(len(nid), dtype=np.int64)
    b0, bm, bb = bases
    in_a = r < WIN
    in_m = (r >= bm) & (r < bm + WIN)
    in_b = r >= bb
    cls = in_a * 1 + in_m * 2 + in_b * 4  # 1,3,7,6,4
    nmax = int(nid.max()) + 1
    cnt = np.bincount(nid * 8 + cls, minlength=nmax * 8).reshape(nmax, 8)
    n0, n1, n2, n3, n4 = cnt[:, 1], cnt[:, 3], cnt[:, 7], cnt[:, 6], cnt[:, 4]
    K = n0 + n1 + n2 + n3 + n4
    ideal = (K + 2) // 3
    t1 = np.clip(ideal - n0, 0, n1)  # {A,M} slots sent to A
    aA = n0 + t1
    t3 = np.clip(ideal - n4, 0, n3)  # {M,B} slots sent to B
    aB = n4 + t3
    addA = np.clip(ideal - aA, 0, n2)  # {A,M,B} slots sent to A
    rem = n2 - addA
    addB = np.clip(ideal - aB, 0, rem)

    q = _cumcount(nid * 8 + cls)
    win = np.full(len(nid), 1, dtype=np.int64)  # default M
    win[cls == 1] = 0
    win[cls == 4] = 2
    m = cls == 3
    win[m] = np.where(q[m] < t1[nid[m]], 0, 1)
    m = cls == 6
    win[m] = np.where(q[m] < t3[nid[m]], 2, 1)
    m = cls == 7
    qa = q[m]
    na = nid[m]
    win[m] = np.where(qa < addA[na], 0, np.where(qa < addA[na] + addB[na], 2, 1))
    # sanity: every slot is in a window that contains its row
    ok = (r >= np.array(bases)[win]) & (r < np.array(bases)[win] + WIN)
    assert ok.all(), "window assignment produced out-of-range slot"
    return win


def _build_layer_tables(slot_nid, slot_r, bases, zero_rows, t_of, p_of, c_of, tiles):
    """Build per-core int16 idx arrays + static shape info for one layer.

    Returns (J [tiles,nwin], Foff [tiles,nwin], F, idx16 [CORES,16,F],
             jpos per slot, joff_by_win [tiles,nwin]).
    """
    nwin = len(bases)
    deg_n = np.bincount(slot_nid, minlength=int(slot_nid.max()) + 1)
    win = _assign_windows(slot_nid, slot_r, deg_n, bases)
    jslot = _cumcount(slot_nid * 4 + win)

    t_slot = t_of[slot_nid]
    J = np.zeros((tiles, nwin), dtype=np.int64)
    np.maximum.at(J, (t_slot, win), jslot + 1)

    # per-(tile,window) idx block offsets (in units of columns of the [16,F] array)
    blk = 8 * J
    Foff = np.zeros((tiles, nwin), dtype=np.int64)
    flat = blk.reshape(-1)
    Foff.reshape(-1)[1:] = np.cumsum(flat)[:-1]
    F = int(flat.sum())

    # prefill with per-window pad value, then scatter real slots
    idx16 = np.empty((CORES, 16, F), dtype=np.int16)
    for t in range(tiles):
        for w in range(nwin):
            if J[t, w] == 0:
                continue
            fo, ln = Foff[t, w], blk[t, w]
            idx16[:, :, fo : fo + ln] = np.int16(zero_rows[w] - bases[w])
    p_slot = p_of[slot_nid]
    c_slot = c_of[slot_nid]
    i = jslot * P + p_slot
    col = Foff[t_slot, win] + i // 16
    row = i % 16
    val = (slot_r - np.array(bases)[win]).astype(np.int16)
    idx16[c_slot, row, col] = val

    # within-tile slot position (for the weight array): [A | M | B] concatenation
    joff_by_win = np.zeros((tiles, nwin), dtype=np.int64)
    for w in range(1, nwin):
        joff_by_win[:, w] = joff_by_win[:, w - 1] + J[:, w - 1]
    jpos = joff_by_win[t_slot, win] + jslot
    return J, Foff, F, idx16, jpos, joff_by_win


def _prep(x, edge_index):
    n = x.shape[0]
    cfg = _cfg(n)
    npc, tiles, jpad, v1, v2 = (
        cfg["npc"],
        cfg["tiles"],
        cfg["jpad"],
        cfg["v1"],
        cfg["v2"],
    )
    assert npc < jpad, "shard needs pad rows (zero-gather targets)"
    src = np.asarray(edge_index[0], dtype=np.int64)
    dst = np.asarray(edge_index[1], dtype=np.int64)
    deg = np.bincount(dst, minlength=n) + 1  # in-degree + self-loop

    order = np.argsort(-deg, kind="stable")  # rank -> node
    rank_of = np.empty(n, dtype=np.int64)
    rank_of[order] = np.arange(n)
    c_of = rank_of % CORES
    j_of = rank_of // CORES
    t_of = j_of // P
    p_of = j_of % P

    # slots: all edges + one self-loop per node, keyed by dst
    slot_nid = np.concatenate([dst, np.arange(n)])
    slot_src = np.concatenate([src, np.arange(n)])

    bases1 = _windows(v1)
    bases2 = _windows(v2)
    zr1 = _zero_rows_l1(cfg, bases1)
    zr2 = _zero_rows_l2(cfg, bases2)

    r1 = _row1(slot_src, cfg)
    r2 = jpad * c_of[slot_src] + j_of[slot_src]

    J1, Foff1, F1, idx16_1, jpos1, joffw1 = _build_layer_tables(
        slot_nid, r1, bases1, zr1, t_of, p_of, c_of, tiles
    )
    J2, Foff2, F2, idx16_2, jpos2, joffw2 = _build_layer_tables(
        slot_nid, r2, bases2, zr2, t_of, p_of, c_of, tiles
    )

    SJ1 = J1.sum(axis=1)  # per-tile total slots

    # deg per natural xpad row (for the device-side dinv⊙x pre-pass);
    # pad rows get 1.0 (their x rows are zero anyway)
    degpad = np.ones(v1, dtype=_np_f32)
    degpad[_row1(np.arange(n), cfg)] = deg
    deg_nat = np.ascontiguousarray(degpad.reshape(v1 // P, P).T)  # [128, v1/128]

    # deg of own (dst) rows, [core][p][tile]; pads get 1e30 so that
    # dinv ~ 1e-15 zeroes their u2 rows (these are the zero-gather targets)
    deg_own = np.full((CORES, P, tiles), 1e30, dtype=_np_f32)
    jgrid = np.arange(jpad)
    tgrid, pgrid = jgrid // P, jgrid % P
    for c in range(CORES):
        valid = jgrid < npc
        nodes = order[np.minimum(jgrid * CORES + c, n - 1)]
        deg_own[c, pgrid[valid], tgrid[valid]] = deg[nodes[valid]]

    # padded x table (layer-1 gather source)
    xpad = np.zeros((v1, IN_C), dtype=_np_f32)
    ns = cfg["n_split"]
    xpad[1 : ns + 1] = x[:ns]
    xpad[ns + 2 : n + 2] = x[ns:]

    shapes = dict(
        cfg=cfg,
        bases1=bases1,
        bases2=bases2,
        J1=J1,
        J2=J2,
        Foff1=Foff1,
        Foff2=Foff2,
        F1=F1,
        F2=F2,
        joffw1=joffw1,
        joffw2=joffw2,
        SJ1=SJ1,
    )
    percore = dict(idx1=idx16_1, idx2=idx16_2, deg_own=deg_own)
    return shapes, percore, xpad, deg_nat, order


def _build(shapes):
    from concourse import bass, bacc, mybir, tile
    from concourse.masks import make_identity

    f32 = mybir.dt.float32
    f16 = mybir.dt.float16
    i16 = mybir.dt.int16
    cfg = shapes["cfg"]
    tiles, jpad, v1, v2 = cfg["tiles"], cfg["jpad"], cfg["v1"], cfg["v2"]
    bases1, bases2 = shapes["bases1"], shapes["bases2"]
    J1, J2 = shapes["J1"], shapes["J2"]
    Foff1, Foff2 = shapes["Foff1"], shapes["Foff2"]
    joffw1, joffw2 = shapes["joffw1"], shapes["joffw2"]
    F1, F2 = shapes["F1"], shapes["F2"]
    SJ1 = shapes["SJ1"]
    v1cols = v1 // P

    nc = bacc.Bacc(None, target_bir_lowering=False, num_swdge_queues=4)
    xpad = nc.declare_dram_parameter("xpad", [v1, IN_C], f32, isOutput=False)
    w1 = nc.declare_dram_parameter("w1", [IN_C, HID_C], f32, isOutput=False)
    w2 = nc.declare_dram_parameter("w2", [HID_C, OUT_C], f32, isOutput=False)
    b1 = nc.declare_dram_parameter("b1", [P, HID_C], f32, isOutput=False)
    b2 = nc.declare_dram_parameter("b2", [P, OUT_C], f32, isOutput=False)
    idx1 = nc.declare_dram_parameter("idx1", [16, F1], i16, isOutput=False)
    idx2 = nc.declare_dram_parameter("idx2", [16, F2], i16, isOutput=False)
    deg_nat = nc.declare_dram_parameter("deg_nat", [P, v1cols], f32, isOutput=False)
    deg_own = nc.declare_dram_parameter("deg_own", [P, tiles], f32, isOutput=False)
    zout = nc.declare_dram_parameter("zout", [jpad, OUT_C], f32, isOutput=True)

    mult = mybir.AluOpType.mult
    qctr = [0]

    def bcast_last(ap, cnt):
        return bass.AP(ap.tensor, ap.offset, list(ap.ap) + [[0, cnt]])

    def emit_gathers(g_t, table_handle, base, idx_t, t, w, Jtw, fo, joff, elem):
        j0 = 0
        while j0 < Jtw:
            jc = min(MAX_J_CHUNK, Jtw - j0)
            nc.gpsimd.dma_gather(
                out_ap=g_t[:, joff + j0 : joff + j0 + jc, :],
                in_ap=table_handle[base:, :],
                idxs_ap=idx_t[:, fo + 8 * j0 : fo + 8 * (j0 + jc)],
                num_idxs=P * jc,
                num_idxs_reg=P * jc,
                elem_size=elem,
                single_packet=False,
                queue_num=qctr[0] % 4,
            )
            qctr[0] += 1
            j0 += jc

    def tree_reduce(g_t, Jtot, agg_f32, down_to=1):
        """Fold the slot dim in place down to `down_to`, then sum the
        remaining slots into the fp32 agg tile."""
        J = Jtot
        while J > down_to:
            a = J // 2
            nc.vector.tensor_add(
                out=g_t[:, 0:a, :], in0=g_t[:, 0:a, :], in1=g_t[:, J - a : J, :]
            )
            J -= a
        if J >= 2:
            nc.vector.tensor_add(out=agg_f32[:], in0=g_t[:, 0, :], in1=g_t[:, 1, :])
            for j in range(2, J):
                nc.vector.tensor_add(out=agg_f32[:], in0=agg_f32[:], in1=g_t[:, j, :])
        else:
            nc.vector.tensor_copy(agg_f32[:], g_t[:, 0, :])

    with tile.TileContext(nc) as tc:
        with (
            tc.tile_pool(name="const", bufs=1) as cp,
            tc.tile_pool(name="dram", bufs=1, space="DRAM") as dp,
        ):
            w1_t = cp.tile([IN_C, HID_C], f32)
            nc.sync.dma_start(w1_t[:], w1[:])
            w2_t = cp.tile([HID_C, OUT_C], f32)
            nc.sync.dma_start(w2_t[:], w2[:])
            b1_t = cp.tile([P, HID_C], f32)
            nc.sync.dma_start(b1_t[:], b1[:])
            b2_t = cp.tile([P, OUT_C], f32)
            nc.sync.dma_start(b2_t[:], b2[:])

            ident_t = cp.tile([P, P], f32)
            make_identity(nc, ident_t[:])

            deg_t = cp.tile([P, tiles], f32)
            nc.sync.dma_start(deg_t[:], deg_own[:])
            dinv_t = cp.tile([P, tiles], f32)
            nc.vector.reciprocal(dinv_t[:], deg_t[:])
            nc.scalar.activation(
                dinv_t[:], dinv_t[:], mybir.ActivationFunctionType.Sqrt
            )

            degn_t = cp.tile([P, v1cols], f32)
            nc.sync.dma_start(degn_t[:], deg_nat[:])
            dinvn_t = cp.tile([P, v1cols], f32)
            nc.vector.reciprocal(dinvn_t[:], degn_t[:])
            nc.scalar.activation(
                dinvn_t[:], dinvn_t[:], mybir.ActivationFunctionType.Sqrt
            )

            u2shard = dp.tile([jpad, HID_C], f32)
            table2 = dp.tile([v2, HID_C], f32)
            xs = dp.tile([v1, IN_C], f16)  # dinv⊙x, fp16 — layer-1 gather table

            # ---- pre-pass: xs = fp16(dinv ⊙ xpad), slab-wise ----
            SLAB = 8  # row-tiles per slab
            with tc.tile_pool(name="pre", bufs=3) as prep_pool:
                s = 0
                while s < v1cols:
                    sc = min(SLAB, v1cols - s)
                    r0 = s * P
                    xt = prep_pool.tile([P, SLAB, IN_C], f32, tag="xt")
                    nc.sync.dma_start(
                        xt[:, :sc, :],
                        xpad[r0 : r0 + sc * P, :].rearrange("(a p) f -> p a f", p=P),
                    )
                    xsb = prep_pool.tile([P, SLAB, IN_C], f16, tag="xsb")
                    dsl = dinvn_t[:, s : s + sc]
                    nc.vector.tensor_tensor(
                        out=xsb[:, :sc, :],
                        in0=xt[:, :sc, :],
                        in1=bcast_last(dsl, IN_C),
                        op=mult,
                    )
                    nc.sync.dma_start(
                        xs[r0 : r0 + sc * P, :].rearrange("(a p) f -> p a f", p=P),
                        xsb[:, :sc, :],
                    )
                    s += sc

            # ---- layer 1 + epilogue ----
            with (
                tc.tile_pool(name="idx1p", bufs=1) as ip1,
                tc.tile_pool(name="g1", bufs=5) as gp1,
                tc.tile_pool(name="l1s", bufs=4) as sp1,
                tc.tile_pool(name="ps", bufs=2, space="PSUM") as pp,
            ):
                idx1_t = ip1.tile([P, F1], i16)
                for g in range(8):
                    nc.sync.dma_start(idx1_t[16 * g : 16 * (g + 1), :], idx1[:, :])

                for t in range(tiles):
                    Jtot = int(SJ1[t])
                    g_t = gp1.tile([P, Jtot, IN_C], f16, tag="g1")
                    for w in range(len(bases1)):
                        if J1[t, w] == 0:
                            continue
                        emit_gathers(
                            g_t,
                            xs,
                            bases1[w],
                            idx1_t,
                            t,
                            w,
                            int(J1[t, w]),
                            int(Foff1[t, w]),
                            int(joffw1[t, w]),
                            IN_C,
                        )
                    agg_t = sp1.tile([P, IN_C], f32, tag="agg")
                    tree_reduce(g_t, Jtot, agg_t, down_to=4)
                    psA = pp.tile([IN_C, P], f32, tag="psA")
                    nc.tensor.transpose(psA[:], agg_t[:], ident_t[:])
                    aggT = sp1.tile([IN_C, P], f32, tag="aggT")
                    nc.vector.tensor_copy(aggT[:], psA[:])
                    ps1 = pp.tile([P, HID_C], f32, tag="ps1")
                    nc.tensor.matmul(
                        out=ps1[:], lhsT=aggT[:], rhs=w1_t[:], start=True, stop=True
                    )
                    h = sp1.tile([P, HID_C], f32, tag="h")
                    nc.vector.tensor_tensor(
                        out=h[:],
                        in0=ps1[:],
                        in1=bcast_last(dinv_t[:, t : t + 1], HID_C),
                        op=mult,
                    )
                    nc.vector.tensor_add(out=h[:], in0=h[:], in1=b1_t[:])
                    nc.scalar.activation(h[:], h[:], mybir.ActivationFunctionType.Relu)
                    psT = pp.tile([HID_C, P], f32, tag="psT")
                    nc.tensor.transpose(psT[:], h[:], ident_t[:])
                    hT = sp1.tile([HID_C, P], f32, tag="hT")
                    nc.vector.tensor_copy(hT[:], psT[:])
                    ps2 = pp.tile([P, OUT_C], f32, tag="ps2")
                    nc.tensor.matmul(
                        out=ps2[:], lhsT=hT[:], rhs=w2_t[:], start=True, stop=True
                    )
                    u2 = sp1.tile([P, HID_C], f32, tag="u2")
                    nc.vector.tensor_tensor(
                        out=u2[:],
                        in0=ps2[:],
                        in1=bcast_last(dinv_t[:, t : t + 1], HID_C),
                        op=mult,
                    )
                    nc.sync.dma_start(u2shard[P * t : P * (t + 1), :], u2[:])

            # ---- all-gather u2 shards -> full table ----
            nc.gpsimd.collective_compute(
                "AllGather",
                mybir.AluOpType.bypass,
                replica_groups=[list(range(CORES))],
                ins=[u2shard[:]],
                outs=[table2[:]],
            )

            # ---- layer 2 ----
            with (
                tc.tile_pool(name="idx2p", bufs=1) as ip2,
                tc.tile_pool(name="g2", bufs=5) as gp2,
                tc.tile_pool(name="l2s", bufs=4) as sp2,
            ):
                idx2_t = ip2.tile([P, F2], i16)
                for g in range(8):
                    nc.sync.dma_start(idx2_t[16 * g : 16 * (g + 1), :], idx2[:, :])

                for t in range(tiles):
                    Jtot = int(J2[t].sum())
                    g_t = gp2.tile([P, Jtot, HID_C], f32, tag="g2")
                    for w in range(len(bases2)):
                        if J2[t, w] == 0:
                            continue
                        emit_gathers(
                            g_t,
                            table2,
                            bases2[w],
                            idx2_t,
                            t,
                            w,
                            int(J2[t, w]),
                            int(Foff2[t, w]),
                            int(joffw2[t, w]),
                            HID_C,
                        )
                    agg2_t = sp2.tile([P, HID_C], f32, tag="agg2")
                    tree_reduce(g_t, Jtot, agg2_t, down_to=2)
                    z = sp2.tile([P, OUT_C], f32, tag="z")
                    nc.vector.tensor_tensor(
                        out=z[:],
                        in0=agg2_t[:],
                        in1=bcast_last(dinv_t[:, t : t + 1], OUT_C),
                        op=mult,
                    )
                    nc.vector.tensor_add(out=z[:], in0=z[:], in1=b2_t[:])
                    nc.sync.dma_start(zout[P * t : P * (t + 1), :], z[:])

    nc.finalize()
    return nc


def kernel(x, edge_index, W1, b1, W2, b2):
    from concourse.bass_utils import run_bass_kernel_spmd

    x = np.ascontiguousarray(np.asarray(x, dtype=_np_f32))
    n = x.shape[0]
    shapes, percore, xpad, deg_nat, order = _prep(x, edge_index)
    nc = _build(shapes)

    b1_bc = np.ascontiguousarray(
        np.broadcast_to(np.asarray(b1, _np_f32), (P, HID_C))
    )
    b2_bc = np.ascontiguousarray(
        np.broadcast_to(np.asarray(b2, _np_f32), (P, OUT_C))
    )
    W1a = np.ascontiguousarray(np.asarray(W1, _np_f32))
    W2a = np.ascontiguousarray(np.asarray(W2, _np_f32))

    in_maps = []
    for c in range(CORES):
        in_maps.append(
            {
                "xpad": xpad,
                "w1": W1a,
                "w2": W2a,
                "b1": b1_bc,
                "b2": b2_bc,
                "idx1": np.ascontiguousarray(percore["idx1"][c]),
                "idx2": np.ascontiguousarray(percore["idx2"][c]),
                "deg_nat": deg_nat,
                "deg_own": np.ascontiguousarray(percore["deg_own"][c]),
            }
        )

    res = run_bass_kernel_spmd(nc, in_maps, list(range(CORES)))

    npc = shapes["cfg"]["npc"]
    z = np.empty((n, OUT_C), dtype=_np_f32)
    for c in range(CORES):
        zc = res.results[c]["zout"][:npc]
        nodes = order[np.arange(npc) * CORES + c]
        valid = np.arange(npc) * CORES + c < n
        z[nodes[valid]] = zc[valid]
    return z

